# revision 42
# baseline (speedup 1.0000x reference)
"""Causal self-attention with relative-position bias on 8 trn2 NeuronCores.

Problem (hardcoded): B=8, S=2048, E=512, H=8, DH=64, fp32 in/out.
  qkv = x @ W_qkv + b_qkv ; per-head causal softmax(q k^T/sqrt(DH) + rel_pos) @ v
  out = ctx @ W_out + b_out

Sharding: 8 cores = 2 batch-groups (4 batches each) x 4 head-groups (2 heads
each).  Each core computes a partial output (its 2 heads' contribution to its
4 batches); host sums the 4 head-group partials per batch group and adds
b_out plus the V-bias correction (softmax rows sum to 1, so the V bias
contributes exactly b_v @ W_out to every output row).

Device-side layout ("S^T" trick): scores are computed transposed, [k, q],
via kT/qT tiles of shape [dh, seq] so that P~ = exp(scores^T) * expRel^T is
directly the stationary-side-free operand of the P@V matmul (no transposes
in the inner loop).  The causal mask is folded into expRel on the host
(masked entries are exactly 0).  x and the output travel as f16 to halve
their DMA traffic.  The softmax denominator Z rides the PV matmul as a
ones-column appended to V (M=65); normalization happens during the
PSUM->SBUF evacuation of ctx^T using a broadcast 1/Z via a small DRAM round
trip.  Engine balance: ACT does exp plus half the output evacuation; DVE
does the rel-multiply (diagonal tiles go to gpsimd), qkv evacuations,
recip, ctx; gpsimd issues rel DMAs, multiplies diagonal tiles.
"""
import numpy as np
from contextlib import ExitStack

import concourse.bass as bass
import concourse.tile as tile
from concourse import bacc, mybir
from concourse.bass_utils import run_bass_kernel_spmd

B, S, E, H, DH = 8, 2048, 512, 8, 64
NB = 4            # batches per core
P = 128
NT = S // P       # 16 k-tiles
NQC = S // 512    # 4 q-chunks of 512
F32 = mybir.dt.float32
F32R = mybir.dt.float32r
F16 = mybir.dt.float16
F8 = mybir.dt.float8e4
EXP = mybir.ActivationFunctionType.Exp
DR = mybir.MatmulPerfMode.DoubleRow

_built = None


def _build():
    nc = bacc.Bacc("TRN2", target_bir_lowering=False, debug=False, num_devices=8)

    d_xT = nc.dram_tensor("xT", [NB, E, S], F16, kind="ExternalInput").ap()
    d_wqk = nc.dram_tensor("wqk", [E, 256], F16, kind="ExternalInput").ap()
    d_bqk = nc.dram_tensor("bqk", [2, P], F32, kind="ExternalInput").ap()
    d_wv = nc.dram_tensor("wv", [E, P], F16, kind="ExternalInput").ap()
    d_wout = nc.dram_tensor("wout", [P, E], F16, kind="ExternalInput").ap()
    d_rel = nc.dram_tensor("rel", [2, S, S], F16, kind="ExternalInput").ap()
    d_ident = nc.dram_tensor("ident", [P, P], F16, kind="ExternalInput").ap()
    d_onesv = nc.dram_tensor("onesv", [P, 1], F16, kind="ExternalInput").ap()
    d_out = nc.dram_tensor("out", [NB, S, E], F16, kind="ExternalOutput").ap()

    with tile.TileContext(nc) as tc, ExitStack() as top:
        const = top.enter_context(tc.tile_pool(name="const", bufs=1))
        persist = top.enter_context(tc.tile_pool(name="persist", bufs=1))
        pw = top.enter_context(tc.tile_pool(name="pw", bufs=1))
        ps = top.enter_context(tc.tile_pool(name="ps", bufs=1, space="PSUM"))
        dr = top.enter_context(tc.tile_pool(name="dr", bufs=3, space="DRAM"))

        # ---- constants into SBUF ----
        wqk_sb = const.tile([P, 4, 256], F16, tag="wqk")
        nc.sync.dma_start(wqk_sb[:], d_wqk.rearrange("(eo p) c -> p eo c", p=P))
        wv_sb = const.tile([P, 4, P], F16, tag="wv")
        nc.sync.dma_start(wv_sb[:], d_wv.rearrange("(eo p) c -> p eo c", p=P))
        wout_sb = const.tile([P, E], F16, tag="wout")
        nc.sync.dma_start(wout_sb[:], d_wout)
        bqk_sb = const.tile([P, 2], F32, tag="bqk")
        nc.sync.dma_start(bqk_sb[:], d_bqk.rearrange("r p -> p r"))
        ident_sb = const.tile([P, P], F16, tag="ident")
        nc.sync.dma_start(ident_sb[:], d_ident)

        # ---- persistent per-batch tensors ----
        qT_all = persist.tile([P, NB, S], F16, tag="qT")
        kT_all = persist.tile([P, NB, S], F16, tag="kT")
        v_all = persist.tile([P, NB, NT, 130], F16, tag="v")
        ones_sb = const.tile([P, 1], F16, tag="ones_sb")
        nc.sync.dma_start(ones_sb[:], d_onesv)
        for _bb in range(NB):
            nc.vector.tensor_copy(
                v_all[:, _bb, :, :]
                .rearrange("p j (two c) -> p j two c", two=2)[:, :, :, 64:65],
                ones_sb[:, None, None, :].to_broadcast([P, NT, 2, 1]))

        def phase_a(b):
            """QKV projections for one batch."""
            for tc4 in range(4):
                tsl = slice(tc4 * 512, (tc4 + 1) * 512)
                xt = pw.tile([P, 4, 512], F16, tag="xt", bufs=3,
                             name=f"xt_{b}_{tc4}")
                nc.sync.dma_start(
                    xt[:], d_xT[b].rearrange("(eo p) t -> p eo t", p=P)[:, :, tsl])
                ps_q = ps.tile([P, 512], F32, tag="po", bufs=4,
                               name=f"ps_q_{b}_{tc4}")
                ps_k = ps.tile([P, 512], F32, tag="po", bufs=4,
                               name=f"ps_k_{b}_{tc4}")
                for eo in range(4):
                    st, sp = (eo == 0), (eo == 3)
                    nc.tensor.matmul(ps_q[:], wqk_sb[:, eo, 0:128], xt[:, eo, :],
                                     start=st, stop=sp)
                for eo in range(4):
                    st, sp = (eo == 0), (eo == 3)
                    nc.tensor.matmul(ps_k[:], wqk_sb[:, eo, 128:256], xt[:, eo, :],
                                     start=st, stop=sp)
                with nc.allow_low_precision(reason="f16 PE operands"):
                    nc.vector.tensor_scalar_add(qT_all[:, b, tsl], ps_q[:],
                                                bqk_sb[:, 0:1])
                    nc.vector.tensor_scalar_add(kT_all[:, b, tsl], ps_k[:],
                                                bqk_sb[:, 1:2])
                ps_v = ps.tile([P, 512], F32, tag="po", bufs=4,
                               name=f"ps_v_{b}_{tc4}")
                for eo in range(4):
                    st, sp = (eo == 0), (eo == 3)
                    nc.tensor.matmul(ps_v[:], wv_sb[:, eo, :], xt[:, eo, :],
                                     start=st, stop=sp)
                vt = pw.tile([P, 512], F16, tag="vt", bufs=2,
                             name=f"vt_{b}_{tc4}")
                with nc.allow_low_precision(reason="f16 V operand"):
                    nc.vector.tensor_copy(vt[:], ps_v[:])
                for q4 in range(4):
                    j = tc4 * 4 + q4
                    ps_tr = ps.tile([P, P], F16, tag="s0", bufs=1,
                                    name=f"ps_tr_{b}_{j}")
                    nc.tensor.transpose(ps_tr[:], vt[:, q4 * 128:(q4 + 1) * 128],
                                        ident_sb[:])
                    nc.vector.tensor_copy(
                        v_all[:, b, j, :]
                        .rearrange("p (two c) -> p two c", two=2)[:, :, 0:64],
                        ps_tr[:].rearrange("p (two c) -> p two c", two=2))

        def phase_b(hooks=None):
            # one pass over q-chunks; rel tiles are loaded once per chunk and
            # shared by both batch-pairs, which halves rel DMA traffic and
            # gpsimd issue cost.
            for i in range(NQC):
                njp = 2 * i + 2
                rel = {}
                for jp in range(njp):
                    off = 256 if jp == 2 * i + 1 else 0
                    qsl = slice(i * 512 + off, (i + 1) * 512)
                    for h in range(2):
                        t = pw.tile([P, 2, 512], F16, tag=f"rel{h}", bufs=8,
                                    name=f"rel{h}_{i}_{jp}")
                        nc.gpsimd.dma_start(
                            t[:, :, off:512],
                            d_rel[h, jp * 256:(jp + 1) * 256, qsl]
                            .rearrange("(jj p) q -> p jj q", p=P))
                        rel[(h, jp)] = t
                for bp in range(2):
                    _phase_b_pair(i, njp, rel, bp)
                if hooks and i in hooks:
                    hooks[i]()

        def _phase_b_pair(i, njp, rel, bp):
                bs = (2 * bp, 2 * bp + 1)
                po = {}
                for b in bs:
                    for h in range(2):
                        po[(b, h)] = ps.tile([P, 512], F32, tag="po", bufs=4,
                                             name=f"po_{b}_{h}_{i}")
                # software pipeline: QK+exp+mul for iteration jp run while the
                # PE consumes PV matmuls of iteration jp-1, so the PE stream
                # (4x QK then 4x PV) never waits on the ACT->DVE chain.
                pt = {}

                def emit_pv(jp):
                    off = 256 if jp == 2 * i + 1 else 0
                    for b in bs:
                        for h in range(2):
                            for jj in range(2):
                                j = 2 * jp + jj
                                nc.tensor.matmul(
                                    po[(b, h)][0:65, off:512],
                                    v_all[:, b, j, 65 * h:65 * h + 65],
                                    pt[(b, h, jp)][:, jj, off:512],
                                    start=(jp == 0 and jj == 0),
                                    stop=(jp == njp - 1 and jj == 1))

                for jp in range(njp):
                    off = 256 if jp == 2 * i + 1 else 0
                    qsl = slice(i * 512 + off, (i + 1) * 512)
                    for b in bs:
                        for h in range(2):
                            hsl = slice(64 * h, 64 * h + 64)
                            ps_s = ps.tile([P, 2, 512], F32, tag=f"s{h}", bufs=1,
                                           name=f"s{h}_{b}_{i}_{jp}")
                            for jj in range(2):
                                j = 2 * jp + jj
                                nc.tensor.matmul(
                                    ps_s[:, jj, off:512],
                                    kT_all[hsl, b, j * 128:(j + 1) * 128],
                                    qT_all[hsl, b, qsl],
                                    start=True, stop=True)
                            es = pw.tile([P, 2, 512], F16, tag=f"es{h}", bufs=3,
                                         name=f"es{h}_{b}_{i}_{jp}")
                            nc.scalar.activation(es[:, :, off:512],
                                                 ps_s[:, :, off:512], EXP)
                            ptt = pw.tile([P, 2, 512], F16, tag=f"pt{h}",
                                          bufs=4, name=f"pt{h}_{b}_{i}_{jp}")
                            eng = nc.gpsimd if off else nc.vector
                            eng.tensor_mul(ptt[:, :, off:512], es[:, :, off:512],
                                           rel[(h, jp)][:, :, off:512])
                            pt[(b, h, jp)] = ptt
                    if jp > 0:
                        emit_pv(jp - 1)
                emit_pv(njp - 1)
                for b in bs:
                    ctx = pw.tile([P, 512], F16, tag="ctx", bufs=2,
                                  name=f"ctx_{b}_{i}")
                    rz = pw.tile([1, 1024], F32, tag="rz", bufs=2,
                                 name=f"rz_{b}_{i}")
                    zs = pw.tile([1, 1024], F32, tag="zs", bufs=2,
                                 name=f"zs_{b}_{i}")
                    for h in range(2):
                        nc.vector.tensor_copy(zs[0:1, 512 * h:512 * h + 512],
                                              po[(b, h)][64:65, :])
                    for h in range(2):
                        nc.vector.reciprocal_approx_fast(
                            rz[0:1, 512 * h:512 * h + 512],
                            zs[0:1, 512 * h:512 * h + 512])
                    scr = dr.tile([1, 1024], F32, tag="scr", bufs=3,
                                  name=f"scr_{b}_{i}")
                    nc.sync.dma_start(scr[:], rz[:])
                    rzb = pw.tile([P, 512], F32, tag="rzb", bufs=2,
                                  name=f"rzb_{b}_{i}")
                    for h in range(2):
                        nc.sync.dma_start(
                            out=rzb[64 * h:64 * h + 64, :],
                            in_=scr[0:1, 512 * h:512 * h + 512]
                            .to_broadcast([64, 512]))
                    for h in range(2):
                        with nc.allow_low_precision(reason="f16 ctx"):
                            nc.vector.tensor_mul(ctx[64 * h:64 * h + 64, :],
                                                 po[(b, h)][0:64, :],
                                                 rzb[64 * h:64 * h + 64, :])
                    osb = pw.tile([P, 4, 512], F16, tag="osb", bufs=2,
                                  name=f"osb_{b}_{i}")
                    for qq in range(4):
                        ps_out = ps.tile([P, 512], F32, tag="po", bufs=4,
                                         name=f"ps_out_{b}_{i}_{qq}")
                        nc.tensor.matmul(ps_out[:],
                                         ctx[:, qq * 128:(qq + 1) * 128],
                                         wout_sb[:], start=True, stop=True)
                        with nc.allow_low_precision(reason="f16 output"):
                            if qq < 1:
                                nc.vector.tensor_copy(osb[:, qq, :], ps_out[:])
                            else:
                                nc.scalar.copy(osb[:, qq, :], ps_out[:])
                    isl = slice(i * 512, (i + 1) * 512)
                    nc.sync.dma_start(
                        d_out[b, isl, :].rearrange("(qq p) e -> p qq e", p=P),
                        osb[:])

        for _b in range(NB):
            phase_a(_b)
        phase_b()

    nc.compile()
    return nc


def _get_built():
    global _built
    if _built is None:
        _built = _build()
    return _built


def kernel(x, W_qkv, b_qkv, W_out, b_out, rel_pos, trace=False):
    x = np.asarray(x, dtype=np.float32)
    W_qkv = np.asarray(W_qkv, dtype=np.float32)
    b_qkv = np.asarray(b_qkv, dtype=np.float32)
    W_out = np.asarray(W_out, dtype=np.float32)
    b_out = np.asarray(b_out, dtype=np.float32)
    rel_pos = np.asarray(rel_pos, dtype=np.float32)

    scale = np.float32(DH ** -0.5)
    Wq, Wk, Wv = W_qkv[:, 0:E], W_qkv[:, E:2 * E], W_qkv[:, 2 * E:3 * E]
    bq, bk, bv = b_qkv[0:E], b_qkv[E:2 * E], b_qkv[2 * E:3 * E]

    # host-side shared prep
    xT = [np.ascontiguousarray(x[4 * g:4 * g + 4].transpose(0, 2, 1))
          .astype(np.float16) for g in range(2)]
    expRelT = [np.triu(np.exp(rel_pos[0, h].T)).astype(np.float16)
               for h in range(H)]
    ident = np.eye(P, dtype=np.float16)
    onesv = np.ones((P, 1), dtype=np.float16)

    in_maps = []
    for c in range(8):
        bg, hg = c // 4, c % 4
        h0, h1 = 2 * hg, 2 * hg + 1
        hcol0 = slice(h0 * DH, h0 * DH + DH)
        hcol1 = slice(h1 * DH, h1 * DH + DH)
        wqk = np.concatenate([Wq[:, hcol0] * scale, Wq[:, hcol1] * scale,
                              Wk[:, hcol0], Wk[:, hcol1]],
                             axis=1).astype(np.float16)
        bqk = np.stack([np.concatenate([bq[hcol0] * scale, bq[hcol1] * scale]),
                        np.concatenate([bk[hcol0], bk[hcol1]])]).astype(np.float32)
        wv = np.concatenate([Wv[:, hcol0], Wv[:, hcol1]],
                            axis=1).astype(np.float16)
        wout = np.concatenate([W_out[hcol0.start:hcol0.stop],
                               W_out[hcol1.start:hcol1.stop]]).astype(np.float16)
        rel = np.stack([expRelT[h0], expRelT[h1]])
        in_maps.append({
            "xT": xT[bg],
            "wqk": np.ascontiguousarray(wqk),
            "bqk": np.ascontiguousarray(bqk),
            "wv": np.ascontiguousarray(wv),
            "wout": np.ascontiguousarray(wout),
            "rel": rel,
            "ident": ident,
            "onesv": onesv,
        })

    nc = _get_built()
    res = run_bass_kernel_spmd(nc, in_maps, core_ids=list(range(8)), trace=trace)

    out = np.zeros((B, S, E), dtype=np.float32)
    for c in range(8):
        bg = c // 4
        out[4 * bg:4 * bg + 4] += res.results[c]["out"].astype(np.float32)
    out += b_out + bv @ W_out
    kernel.last_results = res
    return out


# revision 44
# speedup vs baseline: 1.1556x; 1.1556x over previous
"""Causal self-attention with relative-position bias on 8 trn2 NeuronCores.

Problem (hardcoded): B=8, S=2048, E=512, H=8, DH=64, fp32 in/out.
  qkv = x @ W_qkv + b_qkv ; per-head causal softmax(q k^T/sqrt(DH) + rel_pos) @ v
  out = ctx @ W_out + b_out

Sharding: 8 cores = 2 batch-groups (4 batches each) x 4 head-groups (2 heads
each).  Each core computes a partial output (its 2 heads' contribution to its
4 batches); host sums the 4 head-group partials per batch group and adds
b_out plus the V-bias correction (softmax rows sum to 1, so the V bias
contributes exactly b_v @ W_out to every output row).

Device-side layout ("S^T" trick): scores are computed transposed, [k, q],
via kT/qT tiles of shape [dh, seq] so that P~ = exp(scores^T) * expRel^T is
directly the stationary-side-free operand of the P@V matmul (no transposes
in the inner loop).  The causal mask is folded into expRel on the host
(masked entries are exactly 0).  x and the output travel as f16 to halve
their DMA traffic.  The softmax denominator Z rides the PV matmul as a
ones-column appended to V (M=65); normalization happens during the
PSUM->SBUF evacuation of ctx^T using a broadcast 1/Z via a small DRAM round
trip.  Engine balance: ACT does exp plus half the output evacuation; DVE
does the rel-multiply (diagonal tiles go to gpsimd), qkv evacuations,
recip, ctx; gpsimd issues rel DMAs, multiplies diagonal tiles.
"""
import numpy as np
from contextlib import ExitStack

import concourse.bass as bass
import concourse.tile as tile
from concourse import bacc, mybir
from concourse.bass_utils import run_bass_kernel_spmd

B, S, E, H, DH = 8, 2048, 512, 8, 64
NB = 4            # batches per core
P = 128
NT = S // P       # 16 k-tiles
NQC = S // 512    # 4 q-chunks of 512
F32 = mybir.dt.float32
F32R = mybir.dt.float32r
F16 = mybir.dt.float16
F8 = mybir.dt.float8e4
EXP = mybir.ActivationFunctionType.Exp
DR = mybir.MatmulPerfMode.DoubleRow

_built = None


def _build():
    nc = bacc.Bacc("TRN2", target_bir_lowering=False, debug=False, num_devices=8)

    d_xT = nc.dram_tensor("xT", [NB, E, S], F16, kind="ExternalInput").ap()
    d_wqk = nc.dram_tensor("wqk", [E, 256], F16, kind="ExternalInput").ap()
    d_bqk = nc.dram_tensor("bqk", [2, P], F32, kind="ExternalInput").ap()
    d_wv = nc.dram_tensor("wv", [E, P], F16, kind="ExternalInput").ap()
    d_wout = nc.dram_tensor("wout", [P, E], F16, kind="ExternalInput").ap()
    d_rel = nc.dram_tensor("rel", [2, S, S], F16, kind="ExternalInput").ap()
    d_ident = nc.dram_tensor("ident", [P, P], F16, kind="ExternalInput").ap()
    d_onesv = nc.dram_tensor("onesv", [P, 1], F16, kind="ExternalInput").ap()
    d_out = nc.dram_tensor("out", [NB, S, E], F16, kind="ExternalOutput").ap()

    with tile.TileContext(nc) as tc, ExitStack() as top:
        const = top.enter_context(tc.tile_pool(name="const", bufs=1))
        persist = top.enter_context(tc.tile_pool(name="persist", bufs=1))
        pw = top.enter_context(tc.tile_pool(name="pw", bufs=1))
        ps = top.enter_context(tc.tile_pool(name="ps", bufs=1, space="PSUM"))
        dr = top.enter_context(tc.tile_pool(name="dr", bufs=3, space="DRAM"))

        # ---- constants into SBUF ----
        wqk_sb = const.tile([P, 4, 256], F16, tag="wqk")
        nc.sync.dma_start(wqk_sb[:], d_wqk.rearrange("(eo p) c -> p eo c", p=P))
        wv_sb = const.tile([P, 4, P], F16, tag="wv")
        nc.sync.dma_start(wv_sb[:], d_wv.rearrange("(eo p) c -> p eo c", p=P))
        wout_sb = const.tile([P, E], F16, tag="wout")
        nc.sync.dma_start(wout_sb[:], d_wout)
        bqk_sb = const.tile([P, 2], F32, tag="bqk")
        nc.sync.dma_start(bqk_sb[:], d_bqk.rearrange("r p -> p r"))
        ident_sb = const.tile([P, P], F16, tag="ident")
        nc.sync.dma_start(ident_sb[:], d_ident)

        # ---- persistent per-batch tensors ----
        qT_all = persist.tile([P, NB, S], F16, tag="qT")
        kT_all = persist.tile([P, NB, S], F16, tag="kT")
        v_all = persist.tile([P, NB, NT, 130], F16, tag="v")
        ones_sb = const.tile([P, 1], F16, tag="ones_sb")
        nc.sync.dma_start(ones_sb[:], d_onesv)
        for _bb in range(NB):
            nc.vector.tensor_copy(
                v_all[:, _bb, :, :]
                .rearrange("p j (two c) -> p j two c", two=2)[:, :, :, 64:65],
                ones_sb[:, None, None, :].to_broadcast([P, NT, 2, 1]))

        def phase_a(b):
            """QKV projections for one batch."""
            for tc4 in range(4):
                tsl = slice(tc4 * 512, (tc4 + 1) * 512)
                xt = pw.tile([P, 4, 512], F16, tag="xt", bufs=3,
                             name=f"xt_{b}_{tc4}")
                nc.sync.dma_start(
                    xt[:], d_xT[b].rearrange("(eo p) t -> p eo t", p=P)[:, :, tsl])
                ps_q = ps.tile([P, 512], F32, tag="po", bufs=4,
                               name=f"ps_q_{b}_{tc4}")
                ps_k = ps.tile([P, 512], F32, tag="po", bufs=4,
                               name=f"ps_k_{b}_{tc4}")
                for eo in range(4):
                    st, sp = (eo == 0), (eo == 3)
                    nc.tensor.matmul(ps_q[:], wqk_sb[:, eo, 0:128], xt[:, eo, :],
                                     start=st, stop=sp)
                for eo in range(4):
                    st, sp = (eo == 0), (eo == 3)
                    nc.tensor.matmul(ps_k[:], wqk_sb[:, eo, 128:256], xt[:, eo, :],
                                     start=st, stop=sp)
                with nc.allow_low_precision(reason="f16 PE operands"):
                    nc.vector.tensor_scalar_add(qT_all[:, b, tsl], ps_q[:],
                                                bqk_sb[:, 0:1])
                    nc.vector.tensor_scalar_add(kT_all[:, b, tsl], ps_k[:],
                                                bqk_sb[:, 1:2])
                ps_v = ps.tile([P, 512], F32, tag="po", bufs=4,
                               name=f"ps_v_{b}_{tc4}")
                for eo in range(4):
                    st, sp = (eo == 0), (eo == 3)
                    nc.tensor.matmul(ps_v[:], wv_sb[:, eo, :], xt[:, eo, :],
                                     start=st, stop=sp)
                vt = pw.tile([P, 512], F16, tag="vt", bufs=2,
                             name=f"vt_{b}_{tc4}")
                with nc.allow_low_precision(reason="f16 V operand"):
                    nc.vector.tensor_copy(vt[:], ps_v[:])
                for q4 in range(4):
                    j = tc4 * 4 + q4
                    ps_tr = ps.tile([P, P], F16, tag="s0", bufs=1,
                                    name=f"ps_tr_{b}_{j}")
                    nc.tensor.transpose(ps_tr[:], vt[:, q4 * 128:(q4 + 1) * 128],
                                        ident_sb[:])
                    nc.vector.tensor_copy(
                        v_all[:, b, j, :]
                        .rearrange("p (two c) -> p two c", two=2)[:, :, 0:64],
                        ps_tr[:].rearrange("p (two c) -> p two c", two=2))

        def load_rel(i):
            # rel tiles for one q-chunk, loaded once and shared by both
            # batch-pairs (halves rel DMA traffic and gpsimd issue cost).
            njp = 2 * i + 2
            rel = {}
            for jp in range(njp):
                off = 256 if jp == 2 * i + 1 else 0
                qsl = slice(i * 512 + off, (i + 1) * 512)
                for h in range(2):
                    t = pw.tile([P, 2, 512], F16, tag=f"rel{h}", bufs=8,
                                name=f"rel{h}_{i}_{jp}")
                    nc.gpsimd.dma_start(
                        t[:, :, off:512],
                        d_rel[h, jp * 256:(jp + 1) * 256, qsl]
                        .rearrange("(jj p) q -> p jj q", p=P))
                    rel[(h, jp)] = t
            return rel

        def _phase_b_pair(i, njp, rel, bp):
                bs = (2 * bp, 2 * bp + 1)
                po = {}
                for b in bs:
                    for h in range(2):
                        po[(b, h)] = ps.tile([P, 512], F32, tag="po", bufs=4,
                                             name=f"po_{b}_{h}_{i}")
                # software pipeline: QK+exp+mul for iteration jp run while the
                # PE consumes PV matmuls of iteration jp-1, so the PE stream
                # (4x QK then 4x PV) never waits on the ACT->DVE chain.
                pt = {}

                def emit_pv(jp):
                    off = 256 if jp == 2 * i + 1 else 0
                    for b in bs:
                        for h in range(2):
                            for jj in range(2):
                                j = 2 * jp + jj
                                nc.tensor.matmul(
                                    po[(b, h)][0:65, off:512],
                                    v_all[:, b, j, 65 * h:65 * h + 65],
                                    pt[(b, h, jp)][:, jj, off:512],
                                    start=(jp == 0 and jj == 0),
                                    stop=(jp == njp - 1 and jj == 1))

                for jp in range(njp):
                    off = 256 if jp == 2 * i + 1 else 0
                    qsl = slice(i * 512 + off, (i + 1) * 512)
                    for b in bs:
                        for h in range(2):
                            hsl = slice(64 * h, 64 * h + 64)
                            ps_s = ps.tile([P, 2, 512], F32, tag=f"s{h}", bufs=1,
                                           name=f"s{h}_{b}_{i}_{jp}")
                            for jj in range(2):
                                j = 2 * jp + jj
                                nc.tensor.matmul(
                                    ps_s[:, jj, off:512],
                                    kT_all[hsl, b, j * 128:(j + 1) * 128],
                                    qT_all[hsl, b, qsl],
                                    start=True, stop=True)
                            es = pw.tile([P, 2, 512], F16, tag=f"es{h}", bufs=3,
                                         name=f"es{h}_{b}_{i}_{jp}")
                            nc.scalar.activation(es[:, :, off:512],
                                                 ps_s[:, :, off:512], EXP)
                            ptt = pw.tile([P, 2, 512], F16, tag=f"pt{h}",
                                          bufs=4, name=f"pt{h}_{b}_{i}_{jp}")
                            eng = nc.gpsimd if off else nc.vector
                            eng.tensor_mul(ptt[:, :, off:512], es[:, :, off:512],
                                           rel[(h, jp)][:, :, off:512])
                            pt[(b, h, jp)] = ptt
                    if jp > 0:
                        emit_pv(jp - 1)
                emit_pv(njp - 1)
                for b in bs:
                    ctx = pw.tile([P, 512], F16, tag="ctx", bufs=2,
                                  name=f"ctx_{b}_{i}")
                    rz = pw.tile([1, 1024], F32, tag="rz", bufs=2,
                                 name=f"rz_{b}_{i}")
                    zs = pw.tile([1, 1024], F32, tag="zs", bufs=2,
                                 name=f"zs_{b}_{i}")
                    for h in range(2):
                        nc.vector.tensor_copy(zs[0:1, 512 * h:512 * h + 512],
                                              po[(b, h)][64:65, :])
                    for h in range(2):
                        nc.vector.reciprocal_approx_fast(
                            rz[0:1, 512 * h:512 * h + 512],
                            zs[0:1, 512 * h:512 * h + 512])
                    scr = dr.tile([1, 1024], F32, tag="scr", bufs=3,
                                  name=f"scr_{b}_{i}")
                    nc.sync.dma_start(scr[:], rz[:])
                    rzb = pw.tile([P, 512], F32, tag="rzb", bufs=2,
                                  name=f"rzb_{b}_{i}")
                    for h in range(2):
                        nc.sync.dma_start(
                            out=rzb[64 * h:64 * h + 64, :],
                            in_=scr[0:1, 512 * h:512 * h + 512]
                            .to_broadcast([64, 512]))
                    for h in range(2):
                        with nc.allow_low_precision(reason="f16 ctx"):
                            nc.vector.tensor_mul(ctx[64 * h:64 * h + 64, :],
                                                 po[(b, h)][0:64, :],
                                                 rzb[64 * h:64 * h + 64, :])
                    osb = pw.tile([P, 4, 512], F16, tag="osb", bufs=2,
                                  name=f"osb_{b}_{i}")
                    for qq in range(4):
                        ps_out = ps.tile([P, 512], F32, tag="po", bufs=4,
                                         name=f"ps_out_{b}_{i}_{qq}")
                        nc.tensor.matmul(ps_out[:],
                                         ctx[:, qq * 128:(qq + 1) * 128],
                                         wout_sb[:], start=True, stop=True)
                        with nc.allow_low_precision(reason="f16 output"):
                            if qq < 1:
                                nc.vector.tensor_copy(osb[:, qq, :], ps_out[:])
                            else:
                                nc.scalar.copy(osb[:, qq, :], ps_out[:])
                    isl = slice(i * 512, (i + 1) * 512)
                    nc.sync.dma_start(
                        d_out[b, isl, :].rearrange("(qq p) e -> p qq e", p=P),
                        osb[:])

        # Emission order: batch-pair 0 of the first q-chunk runs right after
        # batches 0,1 are projected, so the scalar engine starts exp work
        # ~40us earlier instead of idling through all of phase A.
        phase_a(0)
        phase_a(1)
        rel0 = load_rel(0)
        _phase_b_pair(0, 2, rel0, 0)
        phase_a(2)
        phase_a(3)
        _phase_b_pair(0, 2, rel0, 1)
        for _i in range(1, NQC):
            _rel = load_rel(_i)
            for _bp in range(2):
                _phase_b_pair(_i, 2 * _i + 2, _rel, _bp)

    nc.compile()
    return nc


def _get_built():
    global _built
    if _built is None:
        _built = _build()
    return _built


def kernel(x, W_qkv, b_qkv, W_out, b_out, rel_pos, trace=False):
    x = np.asarray(x, dtype=np.float32)
    W_qkv = np.asarray(W_qkv, dtype=np.float32)
    b_qkv = np.asarray(b_qkv, dtype=np.float32)
    W_out = np.asarray(W_out, dtype=np.float32)
    b_out = np.asarray(b_out, dtype=np.float32)
    rel_pos = np.asarray(rel_pos, dtype=np.float32)

    scale = np.float32(DH ** -0.5)
    Wq, Wk, Wv = W_qkv[:, 0:E], W_qkv[:, E:2 * E], W_qkv[:, 2 * E:3 * E]
    bq, bk, bv = b_qkv[0:E], b_qkv[E:2 * E], b_qkv[2 * E:3 * E]

    # host-side shared prep
    xT = [np.ascontiguousarray(x[4 * g:4 * g + 4].transpose(0, 2, 1))
          .astype(np.float16) for g in range(2)]
    expRelT = [np.triu(np.exp(rel_pos[0, h].T)).astype(np.float16)
               for h in range(H)]
    ident = np.eye(P, dtype=np.float16)
    onesv = np.ones((P, 1), dtype=np.float16)

    in_maps = []
    for c in range(8):
        bg, hg = c // 4, c % 4
        h0, h1 = 2 * hg, 2 * hg + 1
        hcol0 = slice(h0 * DH, h0 * DH + DH)
        hcol1 = slice(h1 * DH, h1 * DH + DH)
        wqk = np.concatenate([Wq[:, hcol0] * scale, Wq[:, hcol1] * scale,
                              Wk[:, hcol0], Wk[:, hcol1]],
                             axis=1).astype(np.float16)
        bqk = np.stack([np.concatenate([bq[hcol0] * scale, bq[hcol1] * scale]),
                        np.concatenate([bk[hcol0], bk[hcol1]])]).astype(np.float32)
        wv = np.concatenate([Wv[:, hcol0], Wv[:, hcol1]],
                            axis=1).astype(np.float16)
        wout = np.concatenate([W_out[hcol0.start:hcol0.stop],
                               W_out[hcol1.start:hcol1.stop]]).astype(np.float16)
        rel = np.stack([expRelT[h0], expRelT[h1]])
        in_maps.append({
            "xT": xT[bg],
            "wqk": np.ascontiguousarray(wqk),
            "bqk": np.ascontiguousarray(bqk),
            "wv": np.ascontiguousarray(wv),
            "wout": np.ascontiguousarray(wout),
            "rel": rel,
            "ident": ident,
            "onesv": onesv,
        })

    nc = _get_built()
    res = run_bass_kernel_spmd(nc, in_maps, core_ids=list(range(8)), trace=trace)

    out = np.zeros((B, S, E), dtype=np.float32)
    for c in range(8):
        bg = c // 4
        out[4 * bg:4 * bg + 4] += res.results[c]["out"].astype(np.float32)
    out += b_out + bv @ W_out
    kernel.last_results = res
    return out
